# revision 44
# baseline (speedup 1.0000x reference)
"""Trainium2 Bass kernel for a 2-layer GRU decoder with Luong attention.

Sharding (8 NeuronCores, SPMD): recurrence + attention + concat replicated
on every core; vocab projection W_s sharded column-wise so each core reads
a distinct 1/8 of W_s and writes a distinct (T, B, V/8) output slice. No
collectives.

Row order is b-major (row = b*T + t) for the batched phases; the
teacher-forced input transform gx1 uses t-major rows (row = t*B + b) so
each recurrence step reads a contiguous 32-partition slice.

Matmul dtypes: float32r (full PE speed at N>=256, ~1e-4 rel err) for
gx1 / GRU gates / vocab projection; bf16 for attention + concat.

Math notes vs the reference:
  - Wa's bias ba shifts all scores of a row by a constant -> softmax
    cancels it -> dropped unconditionally.
  - scores = (X2 @ Wa) . enc  (enc_proj never materialized).
  - GRU biases are zero in setup_inputs(); if nonzero they are folded in
    via K=1 ones-row matmuls (emitted conditionally).
"""

import sys

sys.path.insert(0, "/opt/trn_rl_repo")

from contextlib import ExitStack

import ml_dtypes
import numpy as np

import concourse.tile as tile
from concourse import bacc, mybir
from concourse.bass_utils import run_bass_kernel_spmd

F32 = mybir.dt.float32
F32R = mybir.dt.float32r
BF16 = mybir.dt.bfloat16
AF = mybir.ActivationFunctionType
AX = mybir.AxisListType.X

T, B, H, E, S, L = 32, 32, 512, 512, 128, 2
V = 50257
NCORES = 8
VC = 6284                     # per-core vocab shard (padded; even for f32r)
VPAD = VC * NCORES
TB = T * B                    # 1024
KH = H // 128                 # 4
MT = TB // 128                # 8
SOS = 1
NCHUNK = -(-VC // 512)        # 13 projection column chunks

_CACHE = {}


def _build(gbias: bool, use_mask: bool, sbias: bool = False):
    nc = bacc.Bacc("TRN2", target_bir_lowering=False, debug=False,
                   num_devices=NCORES)

    def din(name, shape, dt):
        return nc.dram_tensor(name, list(shape), dt, kind="ExternalInput").ap()

    xT = din("xT", (E, TB), F32R)                 # embedded tokens, t-major, transposed
    wih1 = din("wih1", (E, 3 * H), F32R)          # Wih[0].T
    whh1 = din("whh1", (H, 3 * H), F32R)
    wih2 = din("wih2", (H, 3 * H), F32R)
    whh2 = din("whh2", (H, 3 * H), F32R)
    wa = din("wa", (H, H), BF16)                  # Wa (scores use X2 @ Wa)
    wcT = din("wcT", (2 * H, H), BF16)            # Wc.T
    bcT = din("bcT", (128, KH), F32)              # bc reshaped (KH,128).T
    wsT = din("wsT", (H, VC), F32R)               # per-core shard of Ws.T
    bs_c = din("bs", (1, VC), F32R)               # per-core shard of bs
    ones_r = din("ones", (1, 128), F32R)
    identf = din("identf", (128, 128), F32)
    identb = din("identb", (128, 128), BF16)
    enc_n = din("enc", (B * S, H), BF16)          # enc, b-major rows
    encT = din("encT", (H, B * S), BF16)          # enc transposed, b-major cols
    h0 = din("h0", (L, B, H), F32)
    h0T = din("h0T", (L, H, B), F32R)
    if gbias:
        bsum1 = din("bsum1", (1, 3 * H), F32R)    # bih1 + [bhh1_r, bhh1_z, 0]
        bsum2 = din("bsum2", (1, 3 * H), F32R)
        bhn1 = din("bhn1", (1, H), F32R)          # bhh1 n-gate
        bhn2 = din("bhn2", (1, H), F32R)
    if use_mask:
        mb = din("mb", (TB, S), F32)              # -1e9 * mask, b-major rows

    out = nc.dram_tensor("out", [T, B, VC], F32, kind="ExternalOutput").ap()
    outh = nc.dram_tensor("outh", [L, B, H], F32, kind="ExternalOutput").ap()

    with tile.TileContext(nc) as tc, ExitStack() as ctx:
        pw = ctx.enter_context(tc.tile_pool(name="pw", bufs=1))
        ptop = ctx.enter_context(tc.tile_pool(name="ptop", bufs=1))
        ph = ctx.enter_context(tc.tile_pool(name="ph", bufs=3))
        pcell = ctx.enter_context(tc.tile_pool(name="pcell", bufs=5))
        # single PSUM pool: tag "mm" (big matmul outs) 6 banks, "tr" 2 banks
        pps = ctx.enter_context(tc.tile_pool(name="pps", bufs=1, space="PSUM"))

        def ps_mm(p=128, w=512):
            return pps.tile([p, w], F32, tag="mm", bufs=6, name="psmm")

        def ps_tr(p=128, w=128, dt=F32):
            return pps.tile([p, w], dt, tag="tr", bufs=2, name="pstr")

        def dma_k(dst, src, kt, width):
            # (kt*128, width) DRAM -> (128, kt, width) SBUF, one DMA per k-tile
            for k in range(kt):
                nc.sync.dma_start(dst[:, k, :], src[k * 128:(k + 1) * 128, :])

        ident_sb = pw.tile([128, 128], F32)
        nc.sync.dma_start(ident_sb[:], identf[:])
        identb_sb = pw.tile([128, 128], BF16)
        nc.sync.dma_start(identb_sb[:], identb[:])
        ones_sb = pw.tile([1, 128], F32R)
        nc.sync.dma_start(ones_sb[:], ones_r[:])

        w1h = pw.tile([128, KH, 3 * H], F32R)
        dma_k(w1h, whh1, KH, 3 * H)
        w2i = pw.tile([128, KH, 3 * H], F32R)
        dma_k(w2i, wih2, KH, 3 * H)
        w2h = pw.tile([128, KH, 3 * H], F32R)
        dma_k(w2h, whh2, KH, 3 * H)
        if gbias:
            bsum1_sb = pw.tile([1, 3 * H], F32R)
            nc.sync.dma_start(bsum1_sb[:], bsum1[:])
            bsum2_sb = pw.tile([1, 3 * H], F32R)
            nc.sync.dma_start(bsum2_sb[:], bsum2[:])
            bhn1_sb = pw.tile([1, H], F32R)
            nc.sync.dma_start(bhn1_sb[:], bhn1[:])
            bhn2_sb = pw.tile([1, H], F32R)
            nc.sync.dma_start(bhn2_sb[:], bhn2[:])

        # persistent big tensors
        x2t = ptop.tile([128, KH, TB], BF16)      # X2.T, b-major cols
        x2t_bt = x2t.rearrange("p k (b t) -> p k b t", t=T)
        cct = ptop.tile([128, KH, TB], F32R)      # concat.T
        attnT = ptop.tile([128, MT, 128], BF16)   # attn.T (s, tb-group)

        with tc.tile_pool(name="pgx", bufs=1) as pgx:
            # two 64-row tiles so per-step slices sit at base partition 0/32
            # (matmul operands only support base 0/32/64)
            gx1a = pgx.tile([64, MT, 3 * H], BF16)
            gx1b = pgx.tile([64, MT, 3 * H], BF16)

            # ---------- phase 1: gx1 = X @ Wih1.T (+bias), t-major rows ------
            with tc.tile_pool(name="pxt", bufs=2) as pxt, \
                 tc.tile_pool(name="pw1", bufs=2) as pw1:
                for m in range(MT):
                    xt_m = pxt.tile([128, KH, 128], F32R, tag="xt")
                    for k in range(KH):
                        nc.sync.dma_start(
                            xt_m[:, k, :],
                            xT[k * 128:(k + 1) * 128, m * 128:(m + 1) * 128])
                    for g in range(3):
                        if m == 0:
                            w1i_g = pw1.tile([128, KH, 512], F32R, tag=f"w1i{g}",
                                             bufs=1)
                            for k in range(KH):
                                nc.sync.dma_start(
                                    w1i_g[:, k, :],
                                    wih1[k * 128:(k + 1) * 128,
                                         g * 512:(g + 1) * 512])
                            if g == 0:
                                w1i_tiles = []
                            w1i_tiles.append(w1i_g)
                        ps = ps_mm()
                        for k in range(KH):
                            nc.tensor.matmul(
                                ps[:], xt_m[:, k, :], w1i_tiles[g][:, k, :],
                                start=(k == 0), stop=(k == KH - 1 and not gbias))
                        if gbias:
                            nc.tensor.matmul(ps[:], ones_sb[:1, :128],
                                             bsum1_sb[:1, g * 512:(g + 1) * 512],
                                             start=False, stop=True)
                        gsl = slice(g * 512, (g + 1) * 512)
                        nc.vector.tensor_copy(gx1a[:, m, gsl], ps[0:64, :])
                        nc.scalar.copy(gx1b[:, m, gsl], ps[64:128, :])

            # ---------- phase 2: GRU recurrence ------------------------------
            h1 = ph.tile([32, H], F32, tag="h1")
            h2 = ph.tile([32, H], F32, tag="h2")
            nc.sync.dma_start(h1[:], h0[0])
            nc.sync.dma_start(h2[:], h0[1])
            h1t = ph.tile([128, KH, 32], F32R, tag="h1t")
            h2t = ph.tile([128, KH, 32], F32R, tag="h2t")
            for k in range(KH):
                nc.sync.dma_start(h1t[:, k, :], h0T[0, k * 128:(k + 1) * 128, :])
                nc.sync.dma_start(h2t[:, k, :], h0T[1, k * 128:(k + 1) * 128, :])

            def gates(ht, w, bias_n, gx_rz=None):
                # layer-1 gates: r/z also accumulate the (bf16) gx1 slice via
                # an identity matmul, so the cell reads a pre-summed psum.
                ps = []
                for g in range(3):
                    p = ps_mm(32, 512)
                    extra = (g < 2 and gx_rz is not None) or \
                            (g == 2 and bias_n is not None)
                    for k in range(KH):
                        nc.tensor.matmul(p[:], ht[:, k, :],
                                         w[:, k, g * 512:(g + 1) * 512],
                                         start=(k == 0),
                                         stop=(k == KH - 1 and not extra))
                    if g < 2 and gx_rz is not None:
                        # identity diag block at the same base partition as
                        # the gx1 slice (matmul needs equal base partitions)
                        gsl, q2 = gx_rz(g)
                        ib = identb_sb[q2 * 32:(q2 + 1) * 32,
                                       q2 * 32:(q2 + 1) * 32]
                        nc.tensor.matmul(p[:], ib, gsl,
                                         start=False, stop=True)
                    elif g == 2 and bias_n is not None:
                        nc.tensor.matmul(p[:], ones_sb[:1, :32], bias_n[:1, :],
                                         start=False, stop=True)
                    ps.append(p)
                return ps

            def cell(gxn_sb, pr, pz, pn, pgxn, h_prev, tag):
                # r/z psums arrive pre-summed (gx folded in on the PE); the
                # n-gate needs gx_n separate: either an SBUF slice (layer 1,
                # bounced via ACT for the base-partition rule) or a psum.
                r = pcell.tile([32, H], F32, tag="c", name="r")
                nc.scalar.activation(r[:], pr[:], AF.Sigmoid)
                z = pcell.tile([32, H], F32, tag="c", name="z")
                nc.scalar.activation(z[:], pz[:], AF.Sigmoid)
                n_ = pcell.tile([32, H], F32, tag="c", name="n_")
                nc.vector.tensor_mul(n_[:], r[:], pn[:])       # r * gh_n (1 psum)
                if gxn_sb is not None:
                    gxb = pcell.tile([32, H], F32, tag="c", name="gxb")
                    nc.scalar.copy(gxb[:], gxn_sb)
                    gxn = gxb[:]
                else:
                    gxn = pgxn[:]
                nc.vector.tensor_add(n_[:], gxn, n_[:])        # + gx_n
                nc.scalar.activation(n_[:], n_[:], AF.Tanh)
                d = pcell.tile([32, H], F32, tag="c", name="d")
                nc.vector.tensor_sub(d[:], h_prev[:], n_[:])
                nc.vector.tensor_mul(d[:], z[:], d[:])
                hn = ph.tile([32, H], F32, tag=tag, name="hn")
                nc.vector.tensor_add(hn[:], n_[:], d[:])
                return hn

            def transpose4(h_sb, tag, x2t_col=None):
                ht = ph.tile([128, KH, 32], F32R, tag=tag, name="ht")
                for k in range(KH):
                    pt = ps_tr(128, 32)
                    nc.tensor.transpose(pt[:], h_sb[:, k * 128:(k + 1) * 128],
                                        ident_sb[:32, :32])
                    nc.vector.tensor_copy(ht[:, k, :], pt[:])
                    if x2t_col is not None:
                        nc.scalar.copy(x2t_bt[:, k, :, x2t_col], pt[:])
                return ht

            def gates2(h1t_, h2t_):
                # layer-2: r/z gates accumulate gx2 (from h1t@w2i) and gh2
                # (from h2t@w2h) into ONE psum; n gate kept separate.
                ps = []
                for g in range(2):
                    p = ps_mm(32, 512)
                    for k in range(KH):
                        nc.tensor.matmul(p[:], h1t_[:, k, :],
                                         w2i[:, k, g * 512:(g + 1) * 512],
                                         start=(k == 0), stop=False)
                    for k in range(KH):
                        last = (k == KH - 1 and not gbias)
                        nc.tensor.matmul(p[:], h2t_[:, k, :],
                                         w2h[:, k, g * 512:(g + 1) * 512],
                                         start=False, stop=last)
                    if gbias:
                        nc.tensor.matmul(p[:], ones_sb[:1, :32],
                                         bsum2_sb[:1, g * 512:(g + 1) * 512],
                                         start=False, stop=True)
                    ps.append(p)
                pin = ps_mm(32, 512)   # gx2 n-gate (+ bih2_n)
                for k in range(KH):
                    nc.tensor.matmul(pin[:], h1t_[:, k, :],
                                     w2i[:, k, 2 * 512:3 * 512],
                                     start=(k == 0), stop=(k == KH - 1 and not gbias))
                if gbias:
                    nc.tensor.matmul(pin[:], ones_sb[:1, :32],
                                     bsum2_sb[:1, 2 * 512:3 * 512],
                                     start=False, stop=True)
                phn = ps_mm(32, 512)   # gh2 n-gate (+ bhh2_n)
                for k in range(KH):
                    nc.tensor.matmul(phn[:], h2t_[:, k, :],
                                     w2h[:, k, 2 * 512:3 * 512],
                                     start=(k == 0), stop=(k == KH - 1 and not gbias))
                if gbias:
                    nc.tensor.matmul(phn[:], ones_sb[:1, :32], bhn2_sb[:1, :],
                                     start=False, stop=True)
                return ps[0], ps[1], pin, phn

            for t in range(T):
                q2 = (t % 4) % 2
                gxt = gx1a if (t % 4) < 2 else gx1b
                gx = lambda g: (gxt[q2 * 32:(q2 + 1) * 32, t // 4,
                                    g * 512:(g + 1) * 512], q2)
                p1 = gates(h1t, w1h, bhn1_sb if gbias else None, gx_rz=gx)
                h1 = cell(gx(2)[0], p1[0], p1[1], p1[2], None, h1, "h1")
                h1t = transpose4(h1, "h1t")
                prz0, prz1, pin, phn = gates2(h1t, h2t)
                h2 = cell(None, prz0, prz1, phn, pin, h2, "h2")
                h2t = transpose4(h2, "h2t", x2t_col=t)

            nc.sync.dma_start(outh[0], h1[:])
            nc.sync.dma_start(outh[1], h2[:])

        # ---------- phase 3: Y = X2 @ Wa -> YT (bf16) ------------------------
        with tc.tile_pool(name="pyt", bufs=1) as pyt:
            yt = pyt.tile([128, KH, TB], BF16)
            with tc.tile_pool(name="py", bufs=2) as py, \
                 tc.tile_pool(name="pwa", bufs=1) as pwa:
                wa_sb = pwa.tile([128, KH, H], BF16)
                dma_k(wa_sb, wa, KH, H)
                for m in range(MT):
                    psy = ps_mm()
                    for k in range(KH):
                        nc.tensor.matmul(psy[:], x2t[:, k, m * 128:(m + 1) * 128],
                                         wa_sb[:, k, :], start=(k == 0),
                                         stop=(k == KH - 1))
                    y_sb = py.tile([128, 512], BF16, tag="y")
                    nc.vector.tensor_copy(y_sb[:], psy[:])
                    for q in range(KH):
                        ptr = ps_tr(128, 128, BF16)
                        nc.tensor.transpose(ptr[:], y_sb[:, q * 128:(q + 1) * 128],
                                            identb_sb[:])
                        nc.vector.tensor_copy(yt[:, q, m * 128:(m + 1) * 128],
                                              ptr[:])

            # ---------- phase 4: scores + softmax + attnT --------------------
            with tc.tile_pool(name="psc", bufs=2) as psc, \
                 tc.tile_pool(name="pet", bufs=1) as pet, \
                 tc.tile_pool(name="psm", bufs=2) as psm:
                et = pet.tile([128, KH, B * S], BF16)
                dma_k(et, encT, KH, B * S)
                for j in range(MT):               # row-tile: b in [4j, 4j+4)
                    sc = psc.tile([128, S], F32, tag="sc")
                    for c in range(4):
                        b = 4 * j + c
                        pss = ps_mm(32, S)
                        for k in range(KH):
                            nc.tensor.matmul(pss[:], yt[:, k, b * T:b * T + 32],
                                             et[:, k, b * S:(b + 1) * S],
                                             start=(k == 0),
                                             stop=(k == KH - 1))
                        if c % 2 == 0:
                            nc.vector.tensor_copy(sc[c * 32:(c + 1) * 32, :],
                                                  pss[:])
                        else:
                            nc.scalar.copy(sc[c * 32:(c + 1) * 32, :], pss[:])
                    if use_mask:
                        mbt = psm.tile([128, S], F32, tag="mb")
                        nc.sync.dma_start(mbt[:], mb[j * 128:(j + 1) * 128, :])
                        nc.vector.tensor_add(sc[:], sc[:], mbt[:])
                    mx = psm.tile([128, 1], F32, tag="mx")
                    nc.vector.reduce_max(mx[:], sc[:], axis=AX)
                    nc.scalar.mul(mx[:], mx[:], -1.0)
                    ex = psm.tile([128, S], F32, tag="ex")
                    nc.scalar.activation(ex[:], sc[:], AF.Exp, bias=mx[:])
                    sm = psm.tile([128, 1], F32, tag="sm")
                    nc.vector.reduce_sum(sm[:], ex[:], axis=AX)
                    nc.vector.reciprocal(sm[:], sm[:])
                    ab = psm.tile([128, S], BF16, tag="ab")
                    nc.vector.tensor_scalar_mul(ab[:], ex[:], sm[:])
                    pat = ps_tr(128, 128, BF16)
                    nc.tensor.transpose(pat[:], ab[:], identb_sb[:])
                    nc.vector.tensor_copy(attnT[:, j, :], pat[:])

        # ---------- phase 5: ctxT -------------------------------------------
        with tc.tile_pool(name="pctx", bufs=1) as pctx:
            ctxT = pctx.tile([128, KH, TB], BF16)
            with tc.tile_pool(name="pen", bufs=1) as pen:
                eb = pen.tile([128, B, H], BF16)
                enc_sb = enc_n.rearrange("(b s) h -> s b h", s=S)
                for bq in range(4):
                    nc.sync.dma_start(eb[:, bq * 8:(bq + 1) * 8, :],
                                      enc_sb[:, bq * 8:(bq + 1) * 8, :])
                for b in range(B):
                    at_b = attnT[:, b // 4, (b % 4) * 32:(b % 4) * 32 + 32]
                    for m in range(KH):
                        pc = ps_mm(128, 32)
                        nc.tensor.matmul(pc[:], eb[:, b, m * 128:(m + 1) * 128],
                                         at_b, start=True, stop=True)
                        if m % 2 == 0:
                            nc.vector.tensor_copy(
                                ctxT[:, m, b * T:b * T + 32], pc[:])
                        else:
                            nc.scalar.copy(
                                ctxT[:, m, b * T:b * T + 32], pc[:])

            # ---------- phase 6: concatT = tanh(Wc @ [ctx; x2] + bc) ---------
            with tc.tile_pool(name="pwc", bufs=1) as pwc:
                wc_sb = pwc.tile([128, 2 * KH, H], BF16)
                dma_k(wc_sb, wcT, 2 * KH, H)
                bc_sb = pwc.tile([128, KH], F32)
                nc.sync.dma_start(bc_sb[:], bcT[:])
                for m in range(KH):
                    for nh in range(2):
                        pcc = ps_mm()
                        for k in range(2 * KH):
                            src = ctxT if k < KH else x2t
                            nc.tensor.matmul(
                                pcc[:], wc_sb[:, k, m * 128:(m + 1) * 128],
                                src[:, k % KH, nh * 512:(nh + 1) * 512],
                                start=(k == 0), stop=(k == 2 * KH - 1))
                        nc.scalar.activation(cct[:, m, nh * 512:(nh + 1) * 512],
                                             pcc[:], AF.Tanh,
                                             bias=bc_sb[:, m:m + 1])

        # ---------- phase 7: vocab projection + bias + store -----------------
        out_bt = out.rearrange("t b v -> b t v")
        with tc.tile_pool(name="pws", bufs=6) as pws, \
             tc.tile_pool(name="pbs", bufs=2) as pbs, \
             tc.tile_pool(name="po", bufs=3) as po:
            # all chunk widths >= 256 and even, so float32r runs at full rate
            cws = [512] * 11 + [326, 326]
            assert sum(cws) == VC
            c0 = 0
            for ch, cw in enumerate(cws):
                wch = pws.tile([128, KH, 512], F32R, tag="ws")
                for k in range(KH):
                    nc.sync.dma_start(wch[:, k, :cw],
                                      wsT[k * 128:(k + 1) * 128, c0:c0 + cw])
                if sbias:
                    bsch = pbs.tile([1, 512], F32R, tag="bs")
                    nc.sync.dma_start(bsch[:, :cw], bs_c[:, c0:c0 + cw])
                for m in range(MT):
                    pso_t = ps_mm()
                    # bias first (K=1 ones row), then accumulate the 4 k-tiles
                    if sbias:
                        nc.tensor.matmul(pso_t[:, :cw], ones_sb[:1, :128],
                                         bsch[:1, :cw], start=True, stop=False)
                    for k in range(KH):
                        nc.tensor.matmul(pso_t[:, :cw],
                                         cct[:, k, m * 128:(m + 1) * 128],
                                         wch[:, k, :cw],
                                         start=(k == 0 and not sbias),
                                         stop=(k == KH - 1))
                    ot = po.tile([128, 512], F32, tag="ot")
                    nc.vector.tensor_copy(ot[:, :cw], pso_t[:, :cw])
                    nc.sync.dma_start(
                        out_bt[4 * m:4 * m + 4, :, c0:c0 + cw], ot[:, :cw])
                c0 += cw

    nc.compile()
    return nc


def kernel(tgt_seqs, decoder_hidden, encoder_outputs, attention_mask,
           emb_W, gru_Wih, gru_Whh, gru_bih, gru_bhh,
           Wa, ba, Wc, bc, Ws, bs):
    f32 = np.float32
    tgt_seqs = np.asarray(tgt_seqs)
    decoder_hidden = np.asarray(decoder_hidden, f32)
    encoder_outputs = np.asarray(encoder_outputs, f32)
    attention_mask = np.asarray(attention_mask)
    emb_W = np.asarray(emb_W, f32)
    gru_Wih = np.asarray(gru_Wih, f32)
    gru_Whh = np.asarray(gru_Whh, f32)
    gru_bih = np.asarray(gru_bih, f32)
    gru_bhh = np.asarray(gru_bhh, f32)
    Wa = np.asarray(Wa, f32)
    Wc = np.asarray(Wc, f32)
    bc = np.asarray(bc, f32)
    Ws = np.asarray(Ws, f32)
    bs = np.asarray(bs, f32)

    gbias = bool(np.any(gru_bih) or np.any(gru_bhh))
    use_mask = bool(np.any(attention_mask))
    sbias = bool(np.any(bs))
    key = (gbias, use_mask, sbias)
    if key not in _CACHE:
        _CACHE[key] = _build(gbias, use_mask, sbias)
    nc = _CACHE[key]

    # host-side prep (layout only; the only compute is the embedding gather)
    tokens = np.concatenate(
        [np.full((B, 1), SOS, tgt_seqs.dtype), tgt_seqs[:, :-1]], axis=1).T  # (T,B)
    X = emb_W[np.asarray(tokens).reshape(-1)]          # (T*B, E) t-major
    xT = np.ascontiguousarray(X.T)                     # (E, TB)
    bf = ml_dtypes.bfloat16
    enc_b16 = np.ascontiguousarray(
        encoder_outputs.transpose(1, 0, 2)).reshape(B * S, H).astype(bf)
    encT16 = np.ascontiguousarray(
        encoder_outputs.transpose(2, 1, 0)).reshape(H, B * S).astype(bf)
    h0T = np.ascontiguousarray(decoder_hidden.transpose(0, 2, 1))  # (L,H,B)

    ws_pad = np.zeros((VPAD, H), f32)
    ws_pad[:V] = Ws
    bs_pad = np.zeros((VPAD,), f32)
    bs_pad[:V] = bs

    common = {
        "xT": xT,
        "wih1": np.ascontiguousarray(gru_Wih[0].T),
        "whh1": np.ascontiguousarray(gru_Whh[0].T),
        "wih2": np.ascontiguousarray(gru_Wih[1].T),
        "whh2": np.ascontiguousarray(gru_Whh[1].T),
        "wa": Wa.astype(bf),
        "wcT": np.ascontiguousarray(Wc.T).astype(bf),
        "bcT": np.ascontiguousarray(bc.reshape(KH, 128).T),
        "ones": np.ones((1, 128), f32),
        "identf": np.eye(128, dtype=f32),
        "identb": np.eye(128, dtype=f32).astype(bf),
        "enc": enc_b16,
        "encT": encT16,
        "h0": decoder_hidden,
        "h0T": h0T,
    }
    if gbias:
        bsum = gru_bih.copy()
        bsum[:, :2 * H] += gru_bhh[:, :2 * H]
        common["bsum1"] = np.ascontiguousarray(bsum[0:1])
        common["bsum2"] = np.ascontiguousarray(bsum[1:2])
        common["bhn1"] = np.ascontiguousarray(gru_bhh[0:1, 2 * H:])
        common["bhn2"] = np.ascontiguousarray(gru_bhh[1:2, 2 * H:])
    if use_mask:
        mbias = np.where(attention_mask, f32(-1e9), f32(0.0))      # (B,S)
        common["mb"] = np.ascontiguousarray(np.repeat(mbias, T, axis=0))

    in_maps = []
    for c in range(NCORES):
        m = dict(common)
        m["wsT"] = np.ascontiguousarray(ws_pad[c * VC:(c + 1) * VC].T)
        m["bs"] = np.ascontiguousarray(bs_pad[c * VC:(c + 1) * VC]).reshape(1, VC)
        in_maps.append(m)

    res = run_bass_kernel_spmd(nc, in_maps, list(range(NCORES)))
    dec = np.concatenate([res.results[c]["out"] for c in range(NCORES)],
                         axis=-1)[:, :, :V]
    h_final = res.results[0]["outh"]
    return dec, h_final
